# revision 4
# baseline (speedup 1.0000x reference)
"""CAMMambaBlock Trainium2 kernel.

Data-parallel over batch: 8 batch elements -> 8 NeuronCores. Each core runs
the full block (LayerNorm -> in_proj -> causal depthwise conv -> SiLU ->
x_proj -> dt softplus -> selective scan -> gating -> out_proj -> residual)
on its own (c=128, L=9216) slice, streaming over L in chunks.

Selective scan: native DVE prefix-scan per state (16 states), chained across
chunks via `initial` APs. B/C coefficients reach all 128 partitions via
DRAM-bounce broadcast DMAs (spread over the sync/scalar/gpsimd queues; these
are cheap and do not contend with the DVE). The u=v*B and y=h*C multiplies
run as bf16 2x-mode DVE tensor_tensor ops, with a few states' worth pushed
to the gpsimd engine for balance. The 16-way y reduction rides gpsimd
software-DGE accumulate DMAs instead of a DVE add tree.
"""
import types
import numpy as np
import ml_dtypes
from contextlib import ExitStack

import bass_rust

import concourse.bass as bass
import concourse.bacc as bacc
import concourse.tile as tile
from concourse import mybir
from concourse.bass_utils import run_bass_kernel_spmd
from concourse.hw_specs import get_activation_tables


def _single_act_table(self):
    """Force every activation onto natural_log_exp_and_others so the
    table-load pass hoists to one load (the greedy per-func picker would
    otherwise alternate sets and reload ~2.7us each time)."""
    if not any(i.opcode == "Activation" for i in self.all_instructions()):
        return
    keep = "natural_log_exp_and_others"
    tables = [(n, (f if n == keep else set()))
              for n, f in get_activation_tables(self.m.arch).items()]
    bass_rust.insert_act_table_loads(self, tables)

F32 = mybir.dt.float32
BF16 = mybir.dt.bfloat16
AF = mybir.ActivationFunctionType
OP = mybir.AluOpType

C = 128           # channels == d_inner == partitions
NSTATE = 16       # SSM state dim
RANK = 8          # dt rank
LN_EPS = 1e-5
DCONV = 4

L_FULL = 96 * 96  # 9216

# states whose u=v*B and y=h*C multiplies run on gpsimd instead of DVE
GP_U = frozenset()
GP_C = frozenset((1, 2, 3, 5, 6, 7, 9, 10, 11, 13, 14, 15))


def build_nc(L, Tc, sub=512):
    """Build the single-core Bass graph (SPMD across cores)."""
    assert L % Tc == 0 and Tc % sub == 0
    nchunk = L // Tc
    nsub = Tc // sub

    nc = bacc.Bacc()
    x_in = nc.declare_dram_parameter("x", [C, L], F32, isOutput=False)
    w_inT = nc.declare_dram_parameter("w_inT", [C, 5 * C], BF16, isOutput=False)
    w_xpT = nc.declare_dram_parameter("w_xpT", [C, RANK + 2 * NSTATE], BF16,
                                      isOutput=False)
    w_dtT = nc.declare_dram_parameter("w_dtT", [RANK, C], BF16, isOutput=False)
    w_outT = nc.declare_dram_parameter("w_outT", [C, C], BF16, isOutput=False)
    # per-partition columns: [ln_w, ln_b, conv_b, dt_b, D, conv_w0..3, eps, -conv_b]
    cols = nc.declare_dram_parameter("cols", [C, 11], F32, isOutput=False)
    a_cols = nc.declare_dram_parameter("a_cols", [C, NSTATE], F32,
                                       isOutput=False)
    y_out = nc.declare_dram_parameter("y", [C, L], F32, isOutput=True)

    with tile.TileContext(nc) as tc, ExitStack() as ctx:
        wpool = ctx.enter_context(tc.tile_pool(name="weights", bufs=1))
        state = ctx.enter_context(tc.tile_pool(name="state", bufs=1))
        io = ctx.enter_context(tc.tile_pool(name="io", bufs=2))
        work = ctx.enter_context(tc.tile_pool(name="work", bufs=2))
        scanp = ctx.enter_context(tc.tile_pool(name="scan", bufs=3))
        bcp = ctx.enter_context(tc.tile_pool(name="bcast", bufs=6))
        ynp = ctx.enter_context(tc.tile_pool(name="yn", bufs=4))
        scr = ctx.enter_context(tc.tile_pool(name="scratch", bufs=1))
        dram = ctx.enter_context(tc.tile_pool(name="dram", bufs=2,
                                              space="DRAM"))
        ps_st = ctx.enter_context(tc.tile_pool(name="ps_st", bufs=1,
                                               space="PSUM"))
        ps_mm = ctx.enter_context(tc.tile_pool(name="ps_mm", bufs=1,
                                               space="PSUM"))

        # ---- weights to SBUF (once) ----
        winT = wpool.tile([C, 5 * C], BF16, tag="winT")
        nc.sync.dma_start(winT[:], w_inT[:])
        wxpT = wpool.tile([C, RANK + 2 * NSTATE], BF16, tag="wxpT")
        nc.sync.dma_start(wxpT[:], w_xpT[:])
        wdtT = wpool.tile([RANK, C], BF16, tag="wdtT")
        nc.sync.dma_start(wdtT[:], w_dtT[:])
        woutT = wpool.tile([C, C], BF16, tag="woutT")
        nc.sync.dma_start(woutT[:], w_outT[:])
        colsb = wpool.tile([C, 11], F32, tag="cols")
        nc.sync.dma_start(colsb[:], cols[:])
        acol = wpool.tile([C, NSTATE], F32, tag="acol")
        nc.sync.dma_start(acol[:], a_cols[:])
        ones_c = wpool.tile([C, C], BF16, tag="ones")
        nc.gpsimd.memset(ones_c[:], 1.0 / C)

        ln_w, ln_b = colsb[:, 0:1], colsb[:, 1:2]
        conv_b, dt_b, d_col = colsb[:, 2:3], colsb[:, 3:4], colsb[:, 4:5]

        # ---- persistent state ----
        carries = []
        for n in range(NSTATE):
            t = state.tile([C, 1], BF16, tag=f"carry{n}")
            carries.append(t)

        dma_engs = [nc.sync, nc.scalar, nc.gpsimd]

        # ---- streaming loop ----
        for k in range(nchunk):
            t0 = k * Tc
            xin = io.tile([C, Tc], F32, tag="xin")
            nc.sync.dma_start(xin[:], x_in[:, t0:t0 + Tc])

            # LayerNorm over channel (partition) dim
            xin_bf = scr.tile([C, Tc], BF16, tag="xinbf")
            nc.gpsimd.dma_start(xin_bf[:], x_in[:, t0:t0 + Tc])
            sq = scr.tile([C, Tc], BF16, tag="sq")
            nc.scalar.activation(sq[:], xin[:], AF.Square)
            un = work.tile([C, Tc + DCONV - 1], BF16, tag="un")
            if k == 0:
                nc.vector.memset(un[:, 0:DCONV - 1], 0.0)
            else:
                nc.vector.tensor_copy(un[:, 0:DCONV - 1],
                                      prev_un[:, Tc:Tc + DCONV - 1])
            prev_un = un
            for j in range(nsub):
                sl = slice(j * sub, (j + 1) * sub)
                mu = ps_st.tile([C, sub], F32, tag="mu")
                nc.tensor.matmul(mu[:], ones_c[:],
                                 xin_bf[:, sl],
                                 start=True, stop=True)
                m2 = ps_st.tile([C, sub], F32, tag="m2")
                nc.tensor.matmul(m2[:], ones_c[:],
                                 sq[:, sl],
                                 start=True, stop=True)
                musq = scr.tile([C, sub], F32, tag="musq")
                nc.scalar.activation(musq[:], mu[:], AF.Square)
                var = scr.tile([C, sub], F32, tag="var")
                nc.vector.tensor_tensor(var[:], m2[:], musq[:], OP.subtract)
                lnv = scr.tile([C, sub], F32, tag="lnv")
                nc.scalar.activation(lnv[:], var[:], AF.Ln, bias=colsb[:, 9:10])
                rstd = scr.tile([C, sub], F32, tag="rstd")
                nc.scalar.activation(rstd[:], lnv[:], AF.Exp, scale=-0.5)
                dmu = scr.tile([C, sub], F32, tag="dmu")
                nc.vector.tensor_tensor(dmu[:], xin[:, sl], mu[:], OP.subtract)
                dmr = scr.tile([C, sub], F32, tag="dmr")
                nc.vector.tensor_tensor(dmr[:], dmu[:], rstd[:], OP.mult)
                nc.vector.tensor_scalar(
                    un[:, DCONV - 1 + j * sub:DCONV - 1 + (j + 1) * sub],
                    dmr[:], ln_w, ln_b, OP.mult, OP.add)

            # in_proj + folded causal conv (4 shifted matmuls accumulate),
            # z -> zs = silu(z); xs = silu(conv + conv_b) via STT
            zs = work.tile([C, Tc], F32, tag="zs")
            xs = work.tile([C, Tc], BF16, tag="xs")
            for j in range(nsub):
                sl = slice(j * sub, (j + 1) * sub)
                xm_ps = ps_mm.tile([C, sub], F32, tag="xm_ps")
                for kk in range(DCONV):
                    nc.tensor.matmul(
                        xm_ps[:], winT[:, kk * C:(kk + 1) * C],
                        un[:, kk + j * sub:kk + j * sub + sub],
                        start=(kk == 0), stop=(kk == DCONV - 1))
                z_ps = ps_mm.tile([C, sub], F32, tag="z_ps")
                nc.tensor.matmul(z_ps[:], winT[:, 4 * C:5 * C],
                                 un[:, DCONV - 1 + j * sub:
                                     DCONV - 1 + j * sub + sub],
                                 start=True, stop=True)
                es1 = scr.tile([C, sub], F32, tag="es1")
                nc.scalar.activation(es1[:], z_ps[:], AF.Exp, scale=-1.0)
                es2 = scr.tile([C, sub], F32, tag="es2")
                nc.scalar.activation(es2[:], es1[:], AF.Ln, bias=1.0)
                sgz = scr.tile([C, sub], F32, tag="sgz")
                nc.scalar.activation(sgz[:], es2[:], AF.Exp, scale=-1.0)
                nc.vector.tensor_tensor(zs[:, sl], z_ps[:], sgz[:], OP.mult)
                # silu(conv + cb): e^{-(x+cb)} -> ln1p -> e^{-.} -> (x+cb)*sg
                ec1 = scr.tile([C, sub], F32, tag="ec1")
                nc.scalar.activation(ec1[:], xm_ps[:], AF.Exp, scale=-1.0,
                                     bias=colsb[:, 10:11])
                ec2 = scr.tile([C, sub], F32, tag="ec2")
                nc.scalar.activation(ec2[:], ec1[:], AF.Ln, bias=1.0)
                sgc = scr.tile([C, sub], F32, tag="ec1b")
                nc.scalar.activation(sgc[:], ec2[:], AF.Exp, scale=-1.0)
                nc.vector.scalar_tensor_tensor(xs[:, sl], xm_ps[:], conv_b,
                                               sgc[:], OP.add, OP.mult)

            # x_proj -> dtr rows + B/C rows (bf16, bounced to DRAM)
            dtr = work.tile([RANK, Tc], BF16, tag="dtr")
            bc = work.tile([2 * NSTATE, Tc], BF16, tag="bc")
            for j in range(nsub):
                sl = slice(j * sub, (j + 1) * sub)
                dbl = ps_mm.tile([RANK + 2 * NSTATE, sub], F32, tag="dbl")
                nc.tensor.matmul(dbl[:], wxpT[:],
                                 xs[:, sl],
                                 start=True, stop=True)
                nc.scalar.copy(bc[:, sl], dbl[0:2 * NSTATE, :])
                nc.scalar.copy(dtr[:, sl],
                               dbl[2 * NSTATE:2 * NSTATE + RANK, :])
            bcd = dram.tile([NSTATE, 2 * Tc], BF16, tag="bcd")
            nc.sync.dma_start(bcd[:], bc[:])

            # dt = softplus(dt_proj @ dtr + dt_b)
            dt_sb = work.tile([C, Tc], F32, tag="dt")
            for j in range(nsub):
                sl = slice(j * sub, (j + 1) * sub)
                dt_ps = ps_mm.tile([C, sub], F32, tag="dt_ps")
                nc.tensor.matmul(dt_ps[:], wdtT[:],
                                 dtr[:, sl],
                                 start=True, stop=True)
                spe = scr.tile([C, sub], F32, tag="spe")
                nc.scalar.activation(spe[:], dt_ps[:], AF.Exp, bias=dt_b)
                nc.scalar.activation(dt_sb[:, sl], spe[:], AF.Ln, bias=1.0)

            # bf16 dt for 2x-mode multiplies
            dt_bf = work.tile([C, Tc], BF16, tag="dtbf")
            nc.scalar.copy(dt_bf[:], dt_sb[:])
            # v = dt * xs (bf16 2x)
            v_bf = work.tile([C, Tc], BF16, tag="v")
            nc.vector.tensor_tensor(v_bf[:], dt_bf[:], xs[:], OP.mult)

            # broadcast B/C rows for all states (prefetched, 3 DMA queues)
            bcrs = []
            for n in range(NSTATE):
                bcr = bcp.tile([C, 2 * Tc], BF16, tag="bcr")
                dma_engs[n % 3].dma_start(
                    bcr[:], bcd[n:n + 1, :].broadcast_to([C, 2 * Tc]))
                bcrs.append(bcr)

            # per-state: dA (scalar), u=v*B, scan (DVE), y_n=h*C, DMA-accum
            yacc = work.tile([C, Tc], BF16, tag="yacc")
            for n in range(NSTATE):
                dA = scanp.tile([C, Tc], BF16, tag="dA")
                nc.scalar.activation(dA[:], dt_bf[:], AF.Exp,
                                     scale=acol[:, n:n + 1])
                brep = bcrs[n][:, 0:Tc]
                crep = bcrs[n][:, Tc:2 * Tc]
                u = scanp.tile([C, Tc], BF16, tag="u")
                if n in GP_U:
                    nc.gpsimd.tensor_tensor(u[:], v_bf[:], brep, OP.mult)
                else:
                    nc.vector.tensor_tensor(u[:], v_bf[:], brep, OP.mult)
                h = scanp.tile([C, Tc], BF16, tag="h")
                init = 0.0 if k == 0 else carries[n][:]
                nc.vector.tensor_tensor_scan(h[:], dA[:], u[:], init,
                                             OP.mult, OP.add)
                nc.scalar.copy(carries[n][:], h[:, Tc - 1:Tc])
                if n == 0:
                    yn = yacc
                else:
                    yn = ynp.tile([C, Tc], BF16, tag="yn")
                if n in GP_C:
                    nc.gpsimd.tensor_tensor(yn[:], h[:], crep, OP.mult)
                else:
                    nc.vector.tensor_tensor(yn[:], h[:], crep, OP.mult)
                if n > 0:
                    nc.gpsimd.dma_start(yacc[:], yn[:], accum_op=OP.add)

            # y = yacc + D*xs ; gate with zs
            y = scr.tile([C, Tc], F32, tag="y")
            nc.vector.scalar_tensor_tensor(y[:], xs[:], d_col, yacc[:],
                                           OP.mult, OP.add)
            yg = scr.tile([C, Tc], BF16, tag="yg")
            nc.vector.tensor_tensor(yg[:], y[:], zs[:], OP.mult)

            # out_proj + residual
            for j in range(nsub):
                sl = slice(j * sub, (j + 1) * sub)
                o_ps = ps_mm.tile([C, sub], F32, tag="o_ps")
                nc.tensor.matmul(o_ps[:], woutT[:],
                                 yg[:, sl],
                                 start=True, stop=True)
                ob = io.tile([C, sub], F32, tag="ob")
                nc.vector.tensor_tensor(ob[:], o_ps[:], xin[:, sl], OP.add)
                nc.sync.dma_start(y_out[:, t0 + j * sub:t0 + (j + 1) * sub],
                                  ob[:])
    nc.insert_act_table_loads = types.MethodType(_single_act_table, nc)
    nc.compile()
    return nc


def prep_weights(ln_w, ln_b, in_proj_w, conv_w, conv_b, x_proj_w,
                 dt_proj_w, dt_proj_b, A_log, D, out_proj_w):
    eps = np.full_like(ln_w, LN_EPS)
    cols = np.stack([ln_w, ln_b, conv_b, dt_proj_b, D,
                     conv_w[:, 0], conv_w[:, 1], conv_w[:, 2], conv_w[:, 3],
                     eps, -conv_b], axis=1).astype(np.float32)
    return {
        "w_inT": np.ascontiguousarray(np.concatenate(
            [in_proj_w[:128].T * conv_w[:, kk][None, :]
             for kk in range(4)] + [in_proj_w[128:].T],
            axis=1).astype(ml_dtypes.bfloat16)),
        "w_xpT": np.ascontiguousarray(
            x_proj_w[[8 + (i // 2) + 16 * (i % 2) for i in range(32)]
                     + list(range(8))].T
            .astype(ml_dtypes.bfloat16)),
        "w_dtT": np.ascontiguousarray(dt_proj_w.T.astype(ml_dtypes.bfloat16)),
        "w_outT": np.ascontiguousarray(
            out_proj_w.T.astype(ml_dtypes.bfloat16)),
        "cols": cols,
        "a_cols": np.ascontiguousarray(-np.exp(A_log.astype(np.float32))),
    }


def kernel(input, ln_w, ln_b, in_proj_w, conv_w, conv_b, x_proj_w,
           dt_proj_w, dt_proj_b, A_log, D, out_proj_w, _run=None):
    input = np.asarray(input, np.float32)
    b, c, H, W = input.shape
    L = H * W
    assert c == C and b == 8
    wts = prep_weights(
        np.asarray(ln_w, np.float32), np.asarray(ln_b, np.float32),
        np.asarray(in_proj_w, np.float32), np.asarray(conv_w, np.float32),
        np.asarray(conv_b, np.float32), np.asarray(x_proj_w, np.float32),
        np.asarray(dt_proj_w, np.float32), np.asarray(dt_proj_b, np.float32),
        np.asarray(A_log, np.float32), np.asarray(D, np.float32),
        np.asarray(out_proj_w, np.float32))
    nc = build_nc(L, 1536, 512)
    in_maps = []
    for i in range(8):
        m = {"x": np.ascontiguousarray(input[i].reshape(c, L))}
        m.update(wts)
        in_maps.append(m)
    run = _run or run_bass_kernel_spmd
    res = run(nc, in_maps, core_ids=list(range(8)))
    out = np.stack([np.asarray(res.results[i]["y"]).reshape(c, H, W)
                    for i in range(8)])
    return out.astype(np.float32)
